# revision 1
# baseline (speedup 1.0000x reference)
"""Fused multi-head attention block (qkv proj + RMSNorm(q,k) + softmax(QK^T)V
+ out proj), tensor-parallel over 8 TRN2 NeuronCores (2 heads per core).

Layout strategy (per core):
  - Host passes xT [D, N] (transposed activations) so every matmul contracts
    along the partition dim with no on-device transposes of x.
  - qkv is computed dim-major: qT/kT/vT [128(=2 heads x 64), N].
  - RMSNorm per token over head_dim (= partition dim here) is done with
    ones-matmuls on the PE (per-head sum of squares -> [2, N]) and a
    broadcast-back matmul, then one DVE scalar_tensor_tensor multiply.
  - scores are computed transposed: sT [tok_k, tok_q]; exp() on ACT; the
    PV matmul contracts tok_k on partitions with V in natural layout
    augmented with a ones column, so row 64 of the PV accumulator is the
    softmax denominator. No max-subtraction is needed: post-RMSNorm
    |q.k|/sqrt(hd) <= sqrt(hd)*max_scale^2 (= 8 with unit scales).
  - out proj contracts 64 head-dims per head (2 accumulating matmuls per
    tile); per-core partial outputs are summed on the host (the TP
    all-reduce) together with bout.
Matmul-feeding tensors use dtype float32r (fp32 storage, reduced-precision
full-rate PE mode); everything else is fp32.
"""

import os

import numpy as np

B, S, D, H = 2, 2048, 1024, 16
HD = D // H            # 64
N = B * S              # 4096 tokens
NCORES = 8
HPC = H // NCORES      # 2 heads per core
PD = HPC * HD          # 128 per-core head dims
EPS = 1e-6
TOK_T = 512            # phase-1/2 token tile (free dim)
KB = 128               # key block (partition dim in PV)
VW = HD + 2            # vaug width (64 v dims + ones col + pad; even for fp32r)

_last_results = None   # test.py introspection (exec_time_ns, profile)
_nc_cache = None


def _build_program():
    global _nc_cache
    if _nc_cache is not None:
        return _nc_cache
    _nc_cache = _build_program_uncached()
    return _nc_cache


def _build_program_uncached():
    import concourse.bacc as bacc
    import concourse.bass as bass
    import concourse.mybir as mybir
    import concourse.tile as tile
    from concourse.masks import make_identity

    f32 = mybir.dt.float32
    f32r = mybir.dt.float32r
    AF = mybir.ActivationFunctionType
    ALU = mybir.AluOpType

    nc = bacc.Bacc(None, target_bir_lowering=False, debug=False)

    xT_h = nc.declare_dram_parameter("xT", [D, N], f32r, isOutput=False)
    Wq_h = nc.declare_dram_parameter("Wq", [D, 3 * PD], f32r, isOutput=False)
    bq_h = nc.declare_dram_parameter("bq", [PD, 3], f32, isOutput=False)
    Wo_h = nc.declare_dram_parameter("Wo", [PD, D], f32r, isOutput=False)
    qs_h = nc.declare_dram_parameter("qs", [PD, 1], f32, isOutput=False)
    ks_h = nc.declare_dram_parameter("ks", [PD, 1], f32, isOutput=False)
    sel2_h = nc.declare_dram_parameter("sel2", [2, 128], f32r, isOutput=False)
    ones2_h = nc.declare_dram_parameter("ones2", [128, 2], f32r, isOutput=False)
    onescol_h = nc.declare_dram_parameter("onescol", [128, HD], f32r, isOutput=False)
    onespad_h = nc.declare_dram_parameter("onespad", [128, 2], f32r, isOutput=False)
    out_h = nc.declare_dram_parameter("outp", [N, D], f32, isOutput=True)

    n_tt = N // TOK_T           # 8 token tiles
    n_kc = D // 128             # 8 contraction chunks for qkv proj
    n_kb = S // KB              # 16 key blocks per batch
    n_qt = S // TOK_T           # 4 query tiles per batch

    with nc.allow_low_precision(reason="fp32r matmul operands"), \
            tile.TileContext(nc) as tc:
        with (
            tc.tile_pool(name="big", bufs=1) as big,
            tc.tile_pool(name="consts", bufs=1) as consts,
        ):
            # ---- persistent SBUF tensors ----
            qnT = big.tile([PD, N], f32r, tag="qnT")
            knT = big.tile([PD, N], f32r, tag="knT")
            vT = big.tile([PD, N], f32, tag="vT")
            # oT with both heads stacked on partitions (h1 arrives via a
            # partition-shifting SBUF->SBUF DMA)
            onT = big.tile([PD, N], f32r, tag="onT")
            # v in natural layout + ones col (+pad): one [128, VW] tile per (b,h,kb)
            vaug = big.tile([KB, B * HPC * n_kb, VW], f32r, tag="vaug")
            Wsb = big.tile([128, n_kc, 3 * PD], f32r, tag="Wsb")
            WoSb = big.tile([PD, D], f32r, tag="WoSb")
            bqSb = consts.tile([PD, 3], f32, tag="bqSb")
            qsSb = consts.tile([PD, 1], f32, tag="qsSb")
            ksSb = consts.tile([PD, 1], f32, tag="ksSb")
            sel2 = consts.tile([2, 128], f32r, tag="sel2")

            Win = Wq_h[:, :].rearrange("(kc p) j -> p kc j", p=128)
            nc.sync.dma_start(out=Wsb[:, 0, :], in_=Win[:, 0, :])
            nc.sync.dma_start(out=Wsb[:, 1:n_kc, :], in_=Win[:, 1:n_kc, :])
            nc.sync.dma_start(out=WoSb, in_=Wo_h[:, :])
            nc.sync.dma_start(out=bqSb, in_=bq_h[:, :])
            nc.sync.dma_start(out=qsSb, in_=qs_h[:, :])
            nc.sync.dma_start(out=ksSb, in_=ks_h[:, :])
            nc.sync.dma_start(out=sel2, in_=sel2_h[:, :])

            # ---- constants ----
            ident = consts.tile([128, HD], f32, tag="ident")
            make_identity(nc, ident[0:HD, :])
            make_identity(nc, ident[HD:2 * HD, :])
            ones2 = consts.tile([128, 2], f32r, tag="ones2")
            nc.sync.dma_start(out=ones2, in_=ones2_h[:, :])
            ones_col = consts.tile([128, HD], f32r, tag="ones_col")
            nc.sync.dma_start(out=ones_col, in_=onescol_h[:, :])
            # fill every vaug [ones, pad] column pair with one broadcast DMA
            nc.sync.dma_start(
                out=vaug[:, :, HD:VW],
                in_=onespad_h[:, :].unsqueeze(1).broadcast_to(
                    [128, B * HPC * n_kb, 2]),
            )
            eps2 = consts.tile([2, 1], f32, tag="eps2")
            nc.vector.memset(eps2, EPS)
            zb = consts.tile([128, 1], f32, tag="zb")
            nc.vector.memset(zb, 0.0)

            # ================= Phase 1: qkvT + RMSNorm =================
            with (
                tc.tile_pool(name="p1x", bufs=3) as p1x,
                tc.tile_pool(name="p1t", bufs=3) as p1t,
                tc.tile_pool(name="p1s", bufs=4) as p1s,
                tc.tile_pool(name="ps_qkv", bufs=3, space=bass.MemorySpace.PSUM) as ps_qkv,
                tc.tile_pool(name="ps_sum", bufs=2, space=bass.MemorySpace.PSUM) as ps_sum,
                tc.tile_pool(name="ps_bc", bufs=2, space=bass.MemorySpace.PSUM) as ps_bc,
                tc.tile_pool(name="ps_tp", bufs=1, space=bass.MemorySpace.PSUM) as ps_tp,
            ):
                for t in range(n_tt):
                    tsl = slice(t * TOK_T, (t + 1) * TOK_T)
                    xt = p1x.tile([128, n_kc, TOK_T], f32r, tag="xt")
                    xin = xT_h[:, tsl].rearrange("(kc p) n -> p kc n", p=128)
                    for kc in range(n_kc):
                        nc.sync.dma_start(out=xt[:, kc, :], in_=xin[:, kc, :])
                    for m in range(3):  # 0=q, 1=k, 2=v
                        ps = ps_qkv.tile([128, TOK_T], f32, tag="ps")
                        for kc in range(n_kc):
                            nc.tensor.matmul(
                                ps,
                                Wsb[:, kc, m * 128:(m + 1) * 128],
                                xt[:, kc, :],
                                start=(kc == 0),
                                stop=(kc == n_kc - 1),
                            )
                        if m == 2:
                            nc.vector.tensor_scalar_add(vT[:, tsl], ps,
                                                        bqSb[:, 2:3])
                            b = t * TOK_T // S
                            for h in range(HPC):
                                for j in range(TOK_T // KB):
                                    tok0 = t * TOK_T + j * KB
                                    kb = (tok0 - b * S) // KB
                                    idx = (b * HPC + h) * n_kb + kb
                                    tp = ps_tp.tile([KB, HD], f32, tag="tp",
                                                    name="tp")
                                    nc.tensor.transpose(
                                        tp,
                                        vT[h * HD:(h + 1) * HD,
                                           tok0:tok0 + KB],
                                        ident[h * HD:(h + 1) * HD, :],
                                    )
                                    nc.scalar.copy(
                                        out=vaug[:, idx, 0:HD], in_=tp)
                            continue
                        raw = p1t.tile([128, TOK_T], f32, tag="raw")
                        nc.vector.tensor_scalar_add(raw, ps, bqSb[:, m:m + 1])
                        sq = p1t.tile([128, TOK_T], f32r, tag="sq")
                        nc.vector.tensor_mul(sq, raw, raw)
                        ssum = ps_sum.tile([2, TOK_T], f32, tag="ssum")
                        nc.tensor.matmul(ssum, ones2[:, :], sq[:, :],
                                         start=True, stop=True)
                        std = p1s.tile([2, TOK_T], f32, tag="std")
                        nc.scalar.activation(out=std, in_=ssum, func=AF.Sqrt,
                                             bias=eps2[:, :], scale=1.0 / HD)
                        rstd = p1s.tile([2, TOK_T], f32r, tag="rstd")
                        nc.vector.reciprocal(rstd, std)
                        bc = ps_bc.tile([128, TOK_T], f32, tag="bc")
                        nc.tensor.matmul(bc, sel2[:, :], rstd[:, :],
                                         start=True, stop=True)
                        dst = qnT if m == 0 else knT
                        sc = qsSb if m == 0 else ksSb
                        nc.vector.scalar_tensor_tensor(
                            out=dst[:, tsl], in0=raw, scalar=sc[:, 0:1], in1=bc,
                            op0=ALU.mult, op1=ALU.mult,
                        )

            # ========= Phase 2: attention + interleaved out-proj =========
            # The two heads' K=64 score matmuls live at PE row-groups 0-1 /
            # 2-3 (base partitions 0 / 64) and run concurrently in the array.
            # Scores for two consecutive key blocks share one 2-bank PSUM
            # tile so each Exp instruction covers 1024 elements. The
            # out-projection for each query tile is emitted right after its
            # normalize, overlapping the attention of later tiles.
            with (
                tc.tile_pool(name="p2p", bufs=4) as p2p,
                tc.tile_pool(name="p2s", bufs=4) as p2s,
                tc.tile_pool(name="p3o", bufs=4) as p3o,
                tc.tile_pool(name="ps_sc", bufs=2, space=bass.MemorySpace.PSUM) as ps_sc,
                tc.tile_pool(name="ps_o", bufs=1, space=bass.MemorySpace.PSUM) as ps_o,
                tc.tile_pool(name="ps_3", bufs=1, space=bass.MemorySpace.PSUM) as ps_3,
                tc.tile_pool(name="ps_b2", bufs=1, space=bass.MemorySpace.PSUM) as ps_b2,
            ):
                for b in range(B):
                    for qt in range(n_qt):
                        q0 = b * S + qt * TOK_T
                        qsl = slice(q0, q0 + TOK_T)
                        po = [ps_o.tile([VW, TOK_T], f32, tag=f"po{h}",
                                        name=f"po{h}") for h in range(HPC)]
                        for kb2 in range(n_kb // 2):
                            for h in range(HPC):
                                hsl = slice(h * HD, (h + 1) * HD)
                                pss = ps_sc.tile([KB, 2, TOK_T], f32,
                                                 tag="pss", name="pss")
                                for j in range(2):
                                    kb = kb2 * 2 + j
                                    k0 = b * S + kb * KB
                                    nc.tensor.matmul(
                                        pss[:, j, :],
                                        knT[hsl, k0:k0 + KB],
                                        qnT[hsl, qsl],
                                        start=True, stop=True,
                                    )
                                pt = p2p.tile([KB, 2, TOK_T], f32r,
                                              tag=f"pt{h}", name=f"pt{h}")
                                nc.scalar.activation(out=pt, in_=pss,
                                                     func=AF.Exp,
                                                     bias=zb[:, :], scale=1.0)
                                for j in range(2):
                                    kb = kb2 * 2 + j
                                    nc.tensor.matmul(
                                        po[h],
                                        vaug[:, (b * HPC + h) * n_kb + kb, :],
                                        pt[:, j, :],
                                        start=(kb == 0),
                                        stop=(kb == n_kb - 1),
                                    )
                        for h in range(HPC):
                            # copy PV accumulator out fast to free its bank
                            ou = p2s.tile([VW, TOK_T], f32, tag="ou")
                            nc.vector.tensor_copy(ou, po[h])
                            rec = p2s.tile([HD + 1, TOK_T], f32r, tag="rec")
                            nc.vector.reciprocal(rec[HD:HD + 1, :],
                                                 ou[HD:HD + 1, :])
                            bc1 = ps_b2.tile([HD, TOK_T], f32, tag="bc1")
                            nc.tensor.matmul(bc1,
                                             ones_col[HD:HD + 1, :],
                                             rec[HD:HD + 1, :],
                                             start=True, stop=True)
                            bc1s = p2s.tile([HD, TOK_T], f32, tag="bc1s")
                            nc.vector.tensor_copy(bc1s, bc1)
                            if h == 0:
                                nc.vector.tensor_mul(onT[0:HD, qsl],
                                                     ou[0:HD, :], bc1s)
                            else:
                                oh1 = p2s.tile([HD, TOK_T], f32r, tag="oh1")
                                nc.vector.tensor_mul(oh1, ou[0:HD, :], bc1s)
                                nc.sync.dma_start(out=onT[HD:PD, qsl],
                                                  in_=oh1)
                        # out-projection for this query tile
                        last_qt = (b == B - 1 and qt == n_qt - 1)
                        for tb in range(q0 // 128, (q0 + TOK_T) // 128):
                            for od in range(D // TOK_T):
                                i3 = tb * 2 + od
                                if last_qt and i3 % 2 == 1:
                                    # PV accumulators are retired; reuse
                                    # their banks to double-buffer the tail
                                    ps3 = ps_o.tile([128, TOK_T], f32,
                                                    tag=f"po{i3 % 4 // 2}",
                                                    name="ps3t")
                                else:
                                    ps3 = ps_3.tile([128, TOK_T], f32,
                                                    tag="ps3", name="ps3")
                                nc.tensor.matmul(
                                    ps3,
                                    onT[:, tb * 128:(tb + 1) * 128],
                                    WoSb[:, od * TOK_T:(od + 1) * TOK_T],
                                    start=True, stop=True,
                                )
                                ot = p3o.tile([128, TOK_T], f32, tag="ot")
                                nc.vector.tensor_copy(ot, ps3)
                                nc.sync.dma_start(
                                    out=out_h[tb * 128:(tb + 1) * 128,
                                              od * TOK_T:(od + 1) * TOK_T],
                                    in_=ot,
                                )

    nc.compile()
    return nc


def kernel(x, Wqkv, bqkv, Wout, bout, q_scale, k_scale):
    global _last_results
    from concourse.bass_utils import run_bass_kernel_spmd

    x = np.asarray(x, dtype=np.float32)
    Wqkv = np.asarray(Wqkv, dtype=np.float32)
    bqkv = np.asarray(bqkv, dtype=np.float32)
    Wout = np.asarray(Wout, dtype=np.float32)
    bout = np.asarray(bout, dtype=np.float32)
    q_scale = np.asarray(q_scale, dtype=np.float32)
    k_scale = np.asarray(k_scale, dtype=np.float32)

    xT = np.ascontiguousarray(x.reshape(N, D).T)
    sel2 = np.zeros((2, 128), dtype=np.float32)
    sel2[0, 0:64] = 1.0
    sel2[1, 64:128] = 1.0
    in_maps = []
    for c in range(NCORES):
        c0 = c * PD
        Wq_s = np.ascontiguousarray(np.concatenate(
            [Wqkv[:, c0:c0 + PD], Wqkv[:, D + c0:D + c0 + PD],
             Wqkv[:, 2 * D + c0:2 * D + c0 + PD]], axis=1))
        bq_s = np.ascontiguousarray(np.stack(
            [bqkv[c0:c0 + PD], bqkv[D + c0:D + c0 + PD],
             bqkv[2 * D + c0:2 * D + c0 + PD]], axis=1))
        Wo_s = np.ascontiguousarray(Wout[c0:c0 + PD, :])
        qs2 = np.ascontiguousarray(np.tile(q_scale, HPC).reshape(PD, 1) / np.sqrt(HD))
        ks2 = np.ascontiguousarray(np.tile(k_scale, HPC).reshape(PD, 1))
        ones2 = np.zeros((128, 2), dtype=np.float32)
        ones2[0:64, 0] = 1.0
        ones2[64:128, 1] = 1.0
        onescol = np.ones((128, HD), dtype=np.float32)
        onespad = np.zeros((128, 2), dtype=np.float32)
        onespad[:, 0] = 1.0
        in_maps.append({"xT": xT, "Wq": Wq_s, "bq": bq_s, "Wo": Wo_s,
                        "qs": qs2.astype(np.float32), "ks": ks2.astype(np.float32),
                        "sel2": sel2, "ones2": ones2, "onescol": onescol,
                        "onespad": onespad})

    nc = _build_program()
    res = run_bass_kernel_spmd(nc, in_maps, core_ids=list(range(NCORES)))
    _last_results = res

    acc = res.results[0]["outp"].astype(np.float32)
    for c in range(1, NCORES):
        acc = acc + res.results[c]["outp"]
    acc = acc + bout
    return acc.reshape(B, S, D).astype(np.float32)

